# revision 8
# baseline (speedup 1.0000x reference)
"""GAT message-passing kernel for Trainium2, 8 NeuronCores — V3 (gather-free).

Layout strategy (dst-sharded, feature-major, host-pregathered):
 - Nodes sorted by in-degree and packed into 392 windows of 128; window k of
   every core uses one shared edge capacity L_k (max in-degree among the 8
   cores' k-th windows, degree-sorted so padding is tiny), making the SPMD
   program identical across cores.
 - Edge slot (window k, dst slot q, lane l) lives at column
   128*prefix(k) + q*L_k + l.  The host ships, per core, the PRE-GATHERED
   source features x[src_e].T (bf16 [128, S]) and edge features
   [edge_attr | pad_flag].T (bf16 [65, S]) in slot order, so the device
   performs NO per-edge dma_gather (the baseline's 925us gpsimd cost) —
   per-edge projections run as stationary-weight bf16 matmuls at
   1 cycle/column on the PE.
 - Per window: PSUM[68, <=512] accumulates two matmuls:
     rows 0:64  xh_src = W @ x_src            (lhsT = [W.T | u_src.T])
     rows 64:68 a_src + a_edge + pad_flag     (lhsT = vT_ext over [ea | flag])
   a_dst is added in "em" space ([128 = 4h x 32 q, L] tiles reached via a
   DRAM-hop shuffle) where lrelu+exp run full-lane; softmax denominators are
   an X-axis reduce; exp weights return via DRAM to feature-major rows and
   are row-replicated (16 DMAs) for the message multiply; the weighted
   message aggregation per dst node is an X-axis segmented reduce.
 - Per-node self-loop terms (PyG GATConv: self edge_attr = per-dst mean of
   incoming edge_attr) are host-precomputed: es = exp(lrelu(alpha_self)) and
   es*xh fold in at the end.  Output is feature-major [64, 6272] per core;
   the host transposes and scatters back to node order.
"""

import math

import numpy as np
import ml_dtypes

NCORES = 8
P = 128
D_IN = 128
H_HEADS = 4
C_OUT = 16
HC = 64
ED_DIM = 64
NEG_SLOPE = 0.2
PAD_FLAG = -10000.0

TRACE = False
LAST_RESULT = None

bf16 = ml_dtypes.bfloat16


ERUN = 5120
NWIN_RUN_MAX = 8


def _runs_of(Ls):
    runs = []
    k = 0
    NW = len(Ls)
    while k < NW:
        L = Ls[k]
        nwin = 1
        while (k + nwin < NW and Ls[k + nwin] == L and nwin < NWIN_RUN_MAX
               and P * L * (nwin + 1) <= ERUN):
            nwin += 1
        runs.append((k, nwin, L))
        k += nwin
    return runs


def _fold_weights(W, W_edge, att_src, att_dst, att_edge):
    H, C = att_src.shape
    D = W.shape[1]
    ED = W_edge.shape[1]
    u_src = np.einsum("hc,hcd->hd", att_src, W.reshape(H, C, D))
    u_dst = np.einsum("hc,hcd->hd", att_dst, W.reshape(H, C, D))
    v = np.einsum("hc,hcd->hd", att_edge, W_edge.reshape(H, C, ED))
    W68 = np.zeros((D, HC + H), np.float32)
    W68[:, :HC] = W.T
    W68[:, HC:] = u_src.T
    vTe = np.zeros((ED + 1, HC + H), np.float32)
    vTe[:ED, HC:] = v.T
    vTe[ED, HC:] = 1.0  # pad-flag passthrough into every head's alpha
    return W68, vTe, u_src, u_dst, v


def _prep(x, src, dst, ea, W, W_edge, att_src, att_dst, att_edge):
    n = x.shape[0]
    W68, vTe, u_src, u_dst, v = _fold_weights(W, W_edge, att_src, att_dst,
                                              att_edge)
    deg = np.bincount(dst, minlength=n).astype(np.int64)
    order = np.argsort(-deg, kind="stable")
    NW = math.ceil(n / (P * NCORES))          # windows per core (49)
    NWIN = NCORES * NW                        # total windows (392)
    NSLOT = NW * P                            # nodes per core (6272)

    # node -> (sorted-window, slot); sorted-window w -> (core w%8, pos w//8)
    pos_of = np.full(NWIN * P, -1, np.int64)  # padded node list in order
    pos_of[: n] = order
    swin = np.arange(NWIN * P) // P
    # L per sorted window = max degree inside = degree of its first node
    L_raw = np.maximum(deg[order[(np.arange(NWIN) * P).clip(max=n - 1)]], 1)
    # unified per-position L: window position k uses L of sorted window 8k
    Ls = tuple(int(L_raw[NCORES * k]) for k in range(NW))
    S = P * sum(Ls)
    base = np.zeros(NW + 1, np.int64)
    np.cumsum([P * L for L in Ls], out=base[1:])

    node_swin = np.empty(n, np.int64)
    node_slot = np.empty(n, np.int64)
    inv = np.argsort(order, kind="stable")    # node -> rank in sorted order
    node_swin = inv // P
    node_slot = inv % P
    node_core = node_swin % NCORES
    node_k = node_swin // NCORES

    # edge lane index within its dst node
    E = src.shape[0]
    eorder = np.argsort(dst, kind="stable")
    counts = np.bincount(dst, minlength=n)
    offs = np.zeros(n + 1, np.int64)
    np.cumsum(counts, out=offs[1:])
    lane = np.empty(E, np.int64)
    lane[eorder] = np.arange(E) - offs[dst[eorder]]

    ek = node_k[dst]
    ecore = node_core[dst]
    Ls_arr = np.array(Ls, np.int64)
    ecol = base[ek] + node_slot[dst] * Ls_arr[ek] + lane
    assert (lane < Ls_arr[ek]).all()

    xbf = x.astype(bf16)
    xh = (x @ W.T).astype(np.float32)                       # [N, 64]
    a_src_self = x @ u_src.T
    a_dst_self = x @ u_dst.T
    cnt = np.maximum(deg, 1).astype(np.float32)
    loop_attr = np.zeros((n, ED_DIM), np.float32)
    np.add.at(loop_attr, dst, ea)
    loop_attr /= cnt[:, None]
    a_edge_self = loop_attr @ v.T
    al_self = a_src_self + a_dst_self + a_edge_self
    al_self = np.where(al_self > 0, al_self, NEG_SLOPE * al_self)
    es = np.exp(al_self).astype(np.float32)                 # [N, 4]
    a_dst_n = (x @ u_dst.T).astype(np.float32)              # [N, 4]

    in_maps = []
    for c in range(NCORES):
        em = ecore == c
        cols = ecol[em]
        xsrc_rows = np.zeros((S, D_IN), bf16)
        xsrc_rows[cols] = xbf[src[em]]
        eat = np.zeros((ED_DIM + 1, S), np.float32)
        eat[ED_DIM, :] = PAD_FLAG
        eat[:ED_DIM, cols] = ea[em].T
        eat[ED_DIM, cols] = 0.0

        # node tables in this core's (k, q) order
        nsel = node_core == c
        nid = np.where(nsel)[0]
        j = node_k[nid] * P + node_slot[nid]                # col 0..NSLOT-1
        runs = _runs_of(Ls)
        run_of_k = np.empty(NW, np.int64)
        ws0_of_k = np.empty(NW, np.int64)
        for r, (k0, nwin, L) in enumerate(runs):
            run_of_k[k0:k0 + nwin] = r
            ws0_of_k[k0:k0 + nwin] = 4 * (np.arange(nwin))
        NR = len(runs)
        esq = np.ones((P, 32 * NR), np.float32)             # pads: es=1
        adq = np.zeros((P, 32 * NR), bf16)
        exh = np.zeros((HC, NSLOT), bf16)
        q32 = j % 32
        sg = (j // 32) % 4
        k_ = j // P
        r_ = run_of_k[k_]
        ws = ws0_of_k[k_] + sg
        for h in range(H_HEADS):
            esq[h * 32 + ws, 32 * r_ + q32] = es[nid, h]
            adq[h * 32 + ws, 32 * r_ + q32] = a_dst_n[nid, h].astype(bf16)
        exh[:, j] = (es[nid].repeat(C_OUT, axis=1) * xh[nid]).T.astype(bf16)
        in_maps.append(dict(
            xsrcT=np.ascontiguousarray(xsrc_rows.T),
            eaTx=eat.astype(bf16),
            adstqm=adq,
            esqm=esq,
            esxh=exh,
            W68=W68.astype(bf16),
            vTe=vTe.astype(bf16),
        ))
    meta = dict(node_core=node_core, node_k=node_k, node_slot=node_slot)
    return Ls, in_maps, meta


def _build_nc(Ls):
    import concourse.bass as bass
    import concourse.tile as tile
    from concourse import bacc, mybir
    from contextlib import ExitStack

    f32 = mybir.dt.float32
    b16 = mybir.dt.bfloat16
    NW = len(Ls)
    S = P * sum(Ls)
    NSLOT = NW * P
    Lmax = Ls[0]
    Emax = P * Lmax
    base = [0]
    for L in Ls:
        base.append(base[-1] + P * L)

    nc = bacc.Bacc("TRN2", target_bir_lowering=False, debug=False,
                   num_devices=NCORES)
    xsrcT = nc.dram_tensor("xsrcT", [D_IN, S], b16, kind="ExternalInput").ap()
    eaTx = nc.dram_tensor("eaTx", [ED_DIM + 1, S], b16,
                          kind="ExternalInput").ap()
    runs = _runs_of(Ls)
    NR = len(runs)
    adstqm = nc.dram_tensor("adstqm", [P, 32 * NR], b16,
                            kind="ExternalInput").ap()
    esqm = nc.dram_tensor("esqm", [P, 32 * NR], f32,
                          kind="ExternalInput").ap()
    esxh = nc.dram_tensor("esxh", [HC, NSLOT], b16, kind="ExternalInput").ap()
    W68 = nc.dram_tensor("W68", [D_IN, HC + 4], b16, kind="ExternalInput").ap()
    vTe = nc.dram_tensor("vTe", [ED_DIM + 1, HC + 4], b16,
                         kind="ExternalInput").ap()
    out = nc.dram_tensor("out", [HC, NSLOT], f32, kind="ExternalOutput").ap()
    araw_s = nc.dram_tensor("araw_s", [4, S], b16).ap()
    w_s = nc.dram_tensor("w_s", [4, S], b16).ap()
    r_s = nc.dram_tensor("r_s", [4, NSLOT], f32).ap()

    with tile.TileContext(nc) as tc, ExitStack() as ctx:
        cpool = ctx.enter_context(tc.tile_pool(name="const", bufs=1))
        xpool = ctx.enter_context(tc.tile_pool(name="xin", bufs=2))
        epool = ctx.enter_context(tc.tile_pool(name="eain", bufs=2))
        cppool = ctx.enter_context(tc.tile_pool(name="cp", bufs=2))
        wbpool = ctx.enter_context(tc.tile_pool(name="wb", bufs=2))
        wfpool = ctx.enter_context(tc.tile_pool(name="wf", bufs=2))
        empool = ctx.enter_context(tc.tile_pool(name="em", bufs=3))
        mpool = ctx.enter_context(tc.tile_pool(name="m", bufs=2))
        pspool = ctx.enter_context(tc.tile_pool(name="ps", bufs=6,
                                                space="PSUM"))

        W68_sb = cpool.tile([D_IN, HC + 4], b16)
        nc.sync.dma_start(W68_sb[:], W68[:])
        vTe_sb = cpool.tile([ED_DIM + 1, HC + 4], b16)
        nc.sync.dma_start(vTe_sb[:], vTe[:])
        adst_sb = cpool.tile([P, 32 * NR], b16)
        nc.sync.dma_start(adst_sb[:], adstqm[:])
        esq_sb = cpool.tile([P, 32 * NR], f32)
        nc.gpsimd.dma_start(esq_sb[:], esqm[:])
        exh_sb = cpool.tile([HC, NSLOT], b16)
        nc.scalar.dma_start(exh_sb[:], esxh[:])
        den_all = cpool.tile([P, 32 * NR], f32)
        num_all = cpool.tile([HC, NSLOT], f32)

        # equal-L runs of consecutive windows (shared with _prep): em space
        # puts (head, window-subgroup) on partitions and (q, l) on the free
        # axis so every shuffle descriptor moves a contiguous 32*L*2B run
        dma_engs = [nc.sync, nc.scalar, nc.gpsimd]
        for ri, (k0, nwin, L) in enumerate(runs):
            Er = P * L * nwin
            b0 = base[k0]
            nw4 = 4 * nwin
            ERX = max(ERUN, Emax)
            cp = cppool.tile([HC + 4, ERX], b16, tag="cp")
            xs = xpool.tile([D_IN, ERX], b16, tag="xs")
            dma_engs[ri % 3].dma_start(xs[:, :Er], xsrcT[:, b0:b0 + Er])
            eat = epool.tile([ED_DIM + 1, ERX], b16, tag="eat")
            dma_engs[(ri + 1) % 3].dma_start(eat[:, :Er], eaTx[:, b0:b0 + Er])
            for k in range(k0, k0 + nwin):
                E = P * L
                bw = base[k]
                off = bw - b0
                ng = math.ceil(E / 512)
                for g in range(ng):
                    c0 = off + g * 512
                    c1 = min(off + E, c0 + 512)
                    ps = pspool.tile([HC + 4, 512], f32)
                    nc.tensor.matmul(out=ps[:, :c1 - c0], lhsT=W68_sb[:],
                                     rhs=xs[:, c0:c1], start=True, stop=False)
                    nc.tensor.matmul(out=ps[:, :c1 - c0], lhsT=vTe_sb[:],
                                     rhs=eat[:, c0:c1], start=False, stop=True)
                    nc.scalar.activation(cp[:, c0:c1], ps[:, :c1 - c0],
                                         mybir.ActivationFunctionType.Copy)
                nc.gpsimd.dma_start(araw_s[:, bw:bw + E],
                                    cp[HC:HC + 4, off:off + E])

            emc = 32 * L   # em free columns; partitions (h, ws)
            nws = 4 * nwin
            aem = empool.tile([P, 32 * Lmax], b16, tag="aem")
            for h in range(H_HEADS):
                dma_engs[h % 3].dma_start(
                    aem[32 * h:32 * h + nws, :emc]
                    .rearrange("w e -> w e"),
                    araw_s[h, b0:b0 + Er].rearrange("(w e) -> w e", e=emc))
            a2 = empool.tile([P, 32 * Lmax], f32, tag="a2")
            nc.vector.tensor_tensor(
                out=a2[:, :emc].rearrange("p (q l) -> p q l", l=L),
                in0=aem[:, :emc].rearrange("p (q l) -> p q l", l=L),
                in1=adst_sb[:, 32 * ri:32 * ri + 32].unsqueeze(2)
                .broadcast_to([P, 32, L]),
                op=mybir.AluOpType.add)
            # lrelu(x) = max(0.2*x, x)
            nc.vector.scalar_tensor_tensor(
                out=a2[:, :emc], in0=a2[:, :emc], scalar=NEG_SLOPE,
                in1=a2[:, :emc], op0=mybir.AluOpType.mult,
                op1=mybir.AluOpType.max)
            wem = empool.tile([P, 32 * Lmax], b16, tag="wem")
            nc.scalar.activation(wem[:, :emc], a2[:, :emc],
                                 mybir.ActivationFunctionType.Exp)
            nc.vector.tensor_reduce(
                out=den_all[:, 32 * ri:32 * ri + 32],
                in_=wem[:, :emc].rearrange("p (q l) -> p q l", l=L),
                axis=mybir.AxisListType.X, op=mybir.AluOpType.add)
            for h in range(H_HEADS):
                dma_engs[(h + 1) % 3].dma_start(
                    w_s[h, b0:b0 + Er].rearrange("(w e) -> w e", e=emc),
                    wem[32 * h:32 * h + nws, :emc])
            wfm = wfpool.tile([4, ERX], b16, tag="wfm")
            dma_engs[(ri + 2) % 3].dma_start(wfm[:, :Er], w_s[:, b0:b0 + Er])
            wb = wbpool.tile([HC, ERX], b16, tag="wb")
            wbv = wb[:].rearrange("(h c) e -> h c e", c=C_OUT)
            for ci in range(C_OUT):
                dma_engs[ci % 3].dma_start(wbv[:, ci, :Er], wfm[:, :Er])
            for k in range(k0, k0 + nwin):
                off = base[k] - b0
                for s in range(4):
                    slr = slice(off + 32 * L * s, off + 32 * L * (s + 1))
                    M = mpool.tile([HC, 32 * Lmax], b16, tag="M")
                    nc.vector.tensor_tensor(out=M[:, :32 * L],
                                            in0=cp[:HC, slr], in1=wb[:, slr],
                                            op=mybir.AluOpType.mult)
                    nc.vector.tensor_reduce(
                        out=num_all[:, P * k + 32 * s:P * k + 32 * (s + 1)],
                        in_=M[:, :32 * L].rearrange("p (q l) -> p q l", l=L),
                        axis=mybir.AxisListType.X, op=mybir.AluOpType.add)

        # ---- close: out = (num + es*xh_self) / (den + es) ----
        nc.vector.tensor_tensor(out=den_all[:], in0=den_all[:], in1=esq_sb[:],
                                op=mybir.AluOpType.add)
        nc.vector.reciprocal(den_all[:], den_all[:])
        for ri, (k0, nwin, L) in enumerate(runs):
            for h in range(H_HEADS):
                dma_engs[(h + ri) % 3].dma_start(
                    r_s[h, P * k0:P * (k0 + nwin)]
                    .rearrange("(w q) -> w q", q=32),
                    den_all[32 * h:32 * h + 4 * nwin,
                            32 * ri:32 * ri + 32])
        rb = cpool.tile([HC, NSLOT], f32)
        rbv = rb[:].rearrange("(h c) e -> h c e", c=C_OUT)
        for ci in range(C_OUT):
            dma_engs[ci % 3].dma_start(rbv[:, ci, :], r_s[:, :])
        nc.vector.tensor_tensor(out=num_all[:], in0=num_all[:], in1=exh_sb[:],
                                op=mybir.AluOpType.add)
        nc.vector.tensor_tensor(out=num_all[:], in0=num_all[:], in1=rb[:],
                                op=mybir.AluOpType.mult)
        nc.gpsimd.dma_start(out[:], num_all[:])

    nc.compile()
    return nc


_NC_CACHE = {}


def _get_nc(Ls):
    if Ls not in _NC_CACHE:
        _NC_CACHE[Ls] = _build_nc(Ls)
    return _NC_CACHE[Ls]


def kernel(**inputs):
    x = np.asarray(inputs["x"], dtype=np.float32)
    ei = np.asarray(inputs["edge_index"])
    ea = np.asarray(inputs["edge_attr"], dtype=np.float32)
    W = np.asarray(inputs["W"], dtype=np.float32)
    W_edge = np.asarray(inputs["W_edge"], dtype=np.float32)
    att_src = np.asarray(inputs["att_src"], dtype=np.float32)
    att_dst = np.asarray(inputs["att_dst"], dtype=np.float32)
    att_edge = np.asarray(inputs["att_edge"], dtype=np.float32)
    bias = np.asarray(inputs["bias"], dtype=np.float32)
    src = ei[0].astype(np.int64)
    dst = ei[1].astype(np.int64)

    Ls, in_maps, meta = _prep(x, src, dst, ea, W, W_edge, att_src, att_dst,
                              att_edge)
    nc = _get_nc(Ls)

    from concourse.bass_utils import run_bass_kernel_spmd
    res = run_bass_kernel_spmd(nc, in_maps, core_ids=list(range(NCORES)),
                               trace=TRACE)
    if TRACE:
        global LAST_RESULT
        LAST_RESULT = res

    n = x.shape[0]
    out = np.empty((n, HC), np.float32)
    nk, ns, ncore = meta["node_k"], meta["node_slot"], meta["node_core"]
    for c in range(NCORES):
        sel = ncore == c
        out[sel] = res.results[c]["out"][:, nk[sel] * P + ns[sel]].T
    return (out + bias[None, :]).astype(np.float32)


# revision 9
# speedup vs baseline: 1.0508x; 1.0508x over previous
"""GAT message-passing kernel for Trainium2, 8 NeuronCores — V3 (gather-free).

Layout strategy (dst-sharded, feature-major, host-pregathered):
 - Nodes sorted by in-degree and packed into 392 windows of 128; window k of
   every core uses one shared edge capacity L_k (max in-degree among the 8
   cores' k-th windows, degree-sorted so padding is tiny), making the SPMD
   program identical across cores.
 - Edge slot (window k, dst slot q, lane l) lives at column
   128*prefix(k) + q*L_k + l.  The host ships, per core, the PRE-GATHERED
   source features x[src_e].T (bf16 [128, S]) and edge features
   [edge_attr | pad_flag].T (bf16 [65, S]) in slot order, so the device
   performs NO per-edge dma_gather (the baseline's 925us gpsimd cost) —
   per-edge projections run as stationary-weight bf16 matmuls at
   1 cycle/column on the PE.
 - Per window: PSUM[68, <=512] accumulates two matmuls:
     rows 0:64  xh_src = W @ x_src            (lhsT = [W.T | u_src.T])
     rows 64:68 a_src + a_edge + pad_flag     (lhsT = vT_ext over [ea | flag])
   a_dst is added in "em" space ([128 = 4h x 32 q, L] tiles reached via a
   DRAM-hop shuffle) where lrelu+exp run full-lane; softmax denominators are
   an X-axis reduce; exp weights return via DRAM to feature-major rows and
   are row-replicated (16 DMAs) for the message multiply; the weighted
   message aggregation per dst node is an X-axis segmented reduce.
 - Per-node self-loop terms (PyG GATConv: self edge_attr = per-dst mean of
   incoming edge_attr) are host-precomputed: es = exp(lrelu(alpha_self)) and
   es*xh fold in at the end.  Output is feature-major [64, 6272] per core;
   the host transposes and scatters back to node order.
"""

import math

import numpy as np
import ml_dtypes

NCORES = 8
P = 128
D_IN = 128
H_HEADS = 4
C_OUT = 16
HC = 64
ED_DIM = 64
NEG_SLOPE = 0.2
PAD_FLAG = -10000.0

TRACE = False
LAST_RESULT = None

bf16 = ml_dtypes.bfloat16


ERUN = 6144
NWIN_RUN_MAX = 8


def _runs_of(Ls):
    runs = []
    k = 0
    NW = len(Ls)
    while k < NW:
        L = Ls[k]
        nwin = 1
        while (k + nwin < NW and Ls[k + nwin] == L and nwin < NWIN_RUN_MAX
               and P * L * (nwin + 1) <= ERUN):
            nwin += 1
        runs.append((k, nwin, L))
        k += nwin
    return runs


def _fold_weights(W, W_edge, att_src, att_dst, att_edge):
    H, C = att_src.shape
    D = W.shape[1]
    ED = W_edge.shape[1]
    u_src = np.einsum("hc,hcd->hd", att_src, W.reshape(H, C, D))
    u_dst = np.einsum("hc,hcd->hd", att_dst, W.reshape(H, C, D))
    v = np.einsum("hc,hcd->hd", att_edge, W_edge.reshape(H, C, ED))
    W68 = np.zeros((D, HC + H), np.float32)
    W68[:, :HC] = W.T
    W68[:, HC:] = u_src.T
    vTe = np.zeros((ED + 1, HC + H), np.float32)
    vTe[:ED, HC:] = v.T
    vTe[ED, HC:] = 1.0  # pad-flag passthrough into every head's alpha
    return W68, vTe, u_src, u_dst, v


def _prep(x, src, dst, ea, W, W_edge, att_src, att_dst, att_edge):
    n = x.shape[0]
    W68, vTe, u_src, u_dst, v = _fold_weights(W, W_edge, att_src, att_dst,
                                              att_edge)
    deg = np.bincount(dst, minlength=n).astype(np.int64)
    order = np.argsort(-deg, kind="stable")
    NW = math.ceil(n / (P * NCORES))          # windows per core (49)
    NWIN = NCORES * NW                        # total windows (392)
    NSLOT = NW * P                            # nodes per core (6272)

    # node -> (sorted-window, slot); sorted-window w -> (core w%8, pos w//8)
    pos_of = np.full(NWIN * P, -1, np.int64)  # padded node list in order
    pos_of[: n] = order
    swin = np.arange(NWIN * P) // P
    # L per sorted window = max degree inside = degree of its first node
    L_raw = np.maximum(deg[order[(np.arange(NWIN) * P).clip(max=n - 1)]], 1)
    # unified per-position L: window position k uses L of sorted window 8k
    Ls = tuple(int(L_raw[NCORES * k]) for k in range(NW))
    S = P * sum(Ls)
    base = np.zeros(NW + 1, np.int64)
    np.cumsum([P * L for L in Ls], out=base[1:])

    node_swin = np.empty(n, np.int64)
    node_slot = np.empty(n, np.int64)
    inv = np.argsort(order, kind="stable")    # node -> rank in sorted order
    node_swin = inv // P
    node_slot = inv % P
    node_core = node_swin % NCORES
    node_k = node_swin // NCORES

    # edge lane index within its dst node
    E = src.shape[0]
    eorder = np.argsort(dst, kind="stable")
    counts = np.bincount(dst, minlength=n)
    offs = np.zeros(n + 1, np.int64)
    np.cumsum(counts, out=offs[1:])
    lane = np.empty(E, np.int64)
    lane[eorder] = np.arange(E) - offs[dst[eorder]]

    ek = node_k[dst]
    ecore = node_core[dst]
    Ls_arr = np.array(Ls, np.int64)
    ecol = base[ek] + node_slot[dst] * Ls_arr[ek] + lane
    assert (lane < Ls_arr[ek]).all()

    xbf = x.astype(bf16)
    xh = (x @ W.T).astype(np.float32)                       # [N, 64]
    a_src_self = x @ u_src.T
    a_dst_self = x @ u_dst.T
    cnt = np.maximum(deg, 1).astype(np.float32)
    loop_attr = np.zeros((n, ED_DIM), np.float32)
    np.add.at(loop_attr, dst, ea)
    loop_attr /= cnt[:, None]
    a_edge_self = loop_attr @ v.T
    al_self = a_src_self + a_dst_self + a_edge_self
    al_self = np.where(al_self > 0, al_self, NEG_SLOPE * al_self)
    es = np.exp(al_self).astype(np.float32)                 # [N, 4]
    a_dst_n = (x @ u_dst.T).astype(np.float32)              # [N, 4]

    in_maps = []
    for c in range(NCORES):
        em = ecore == c
        cols = ecol[em]
        xsrc_rows = np.zeros((S, D_IN), bf16)
        xsrc_rows[cols] = xbf[src[em]]
        eat = np.zeros((ED_DIM + 1, S), np.float32)
        eat[ED_DIM, :] = PAD_FLAG
        eat[:ED_DIM, cols] = ea[em].T
        eat[ED_DIM, cols] = 0.0

        # node tables in this core's (k, q) order
        nsel = node_core == c
        nid = np.where(nsel)[0]
        j = node_k[nid] * P + node_slot[nid]                # col 0..NSLOT-1
        runs = _runs_of(Ls)
        run_of_k = np.empty(NW, np.int64)
        ws0_of_k = np.empty(NW, np.int64)
        for r, (k0, nwin, L) in enumerate(runs):
            run_of_k[k0:k0 + nwin] = r
            ws0_of_k[k0:k0 + nwin] = 4 * (np.arange(nwin))
        NR = len(runs)
        esq = np.ones((P, 32 * NR), np.float32)             # pads: es=1
        adq = np.zeros((P, 32 * NR), bf16)
        exh = np.zeros((HC, NSLOT), bf16)
        q32 = j % 32
        sg = (j // 32) % 4
        k_ = j // P
        r_ = run_of_k[k_]
        ws = ws0_of_k[k_] + sg
        for h in range(H_HEADS):
            esq[h * 32 + ws, 32 * r_ + q32] = es[nid, h]
            adq[h * 32 + ws, 32 * r_ + q32] = a_dst_n[nid, h].astype(bf16)
        exh[:, j] = (es[nid].repeat(C_OUT, axis=1) * xh[nid]).T.astype(bf16)
        in_maps.append(dict(
            xsrcT=np.ascontiguousarray(xsrc_rows.T),
            eaTx=eat.astype(bf16),
            adstqm=adq,
            esqm=esq,
            esxh=exh,
            W68=W68.astype(bf16),
            vTe=vTe.astype(bf16),
        ))
    meta = dict(node_core=node_core, node_k=node_k, node_slot=node_slot)
    return Ls, in_maps, meta


def _build_nc(Ls):
    import concourse.bass as bass
    import concourse.tile as tile
    from concourse import bacc, mybir
    from contextlib import ExitStack

    f32 = mybir.dt.float32
    b16 = mybir.dt.bfloat16
    NW = len(Ls)
    S = P * sum(Ls)
    NSLOT = NW * P
    Lmax = Ls[0]
    Emax = P * Lmax
    base = [0]
    for L in Ls:
        base.append(base[-1] + P * L)

    nc = bacc.Bacc("TRN2", target_bir_lowering=False, debug=False,
                   num_devices=NCORES)
    xsrcT = nc.dram_tensor("xsrcT", [D_IN, S], b16, kind="ExternalInput").ap()
    eaTx = nc.dram_tensor("eaTx", [ED_DIM + 1, S], b16,
                          kind="ExternalInput").ap()
    runs = _runs_of(Ls)
    NR = len(runs)
    adstqm = nc.dram_tensor("adstqm", [P, 32 * NR], b16,
                            kind="ExternalInput").ap()
    esqm = nc.dram_tensor("esqm", [P, 32 * NR], f32,
                          kind="ExternalInput").ap()
    esxh = nc.dram_tensor("esxh", [HC, NSLOT], b16, kind="ExternalInput").ap()
    W68 = nc.dram_tensor("W68", [D_IN, HC + 4], b16, kind="ExternalInput").ap()
    vTe = nc.dram_tensor("vTe", [ED_DIM + 1, HC + 4], b16,
                         kind="ExternalInput").ap()
    out = nc.dram_tensor("out", [HC, NSLOT], f32, kind="ExternalOutput").ap()
    araw_s = nc.dram_tensor("araw_s", [4, S], b16).ap()
    w_s = nc.dram_tensor("w_s", [4, S], b16).ap()
    r_s = nc.dram_tensor("r_s", [4, NSLOT], f32).ap()

    with tile.TileContext(nc) as tc, ExitStack() as ctx:
        cpool = ctx.enter_context(tc.tile_pool(name="const", bufs=1))
        xpool = ctx.enter_context(tc.tile_pool(name="xin", bufs=2))
        epool = ctx.enter_context(tc.tile_pool(name="eain", bufs=2))
        cppool = ctx.enter_context(tc.tile_pool(name="cp", bufs=2))
        wbpool = ctx.enter_context(tc.tile_pool(name="wb", bufs=2))
        wfpool = ctx.enter_context(tc.tile_pool(name="wf", bufs=2))
        empool = ctx.enter_context(tc.tile_pool(name="em", bufs=3))
        mpool = ctx.enter_context(tc.tile_pool(name="m", bufs=2))
        pspool = ctx.enter_context(tc.tile_pool(name="ps", bufs=6,
                                                space="PSUM"))

        W68_sb = cpool.tile([D_IN, HC + 4], b16)
        nc.sync.dma_start(W68_sb[:], W68[:])
        vTe_sb = cpool.tile([ED_DIM + 1, HC + 4], b16)
        nc.sync.dma_start(vTe_sb[:], vTe[:])
        adst_sb = cpool.tile([P, 32 * NR], b16)
        nc.sync.dma_start(adst_sb[:], adstqm[:])
        esq_sb = cpool.tile([P, 32 * NR], f32)
        nc.gpsimd.dma_start(esq_sb[:], esqm[:])
        exh_sb = cpool.tile([HC, NSLOT], b16)
        nc.scalar.dma_start(exh_sb[:], esxh[:])
        den_all = cpool.tile([P, 32 * NR], f32)
        num_all = cpool.tile([HC, NSLOT], f32)

        # equal-L runs of consecutive windows (shared with _prep): em space
        # puts (head, window-subgroup) on partitions and (q, l) on the free
        # axis so every shuffle descriptor moves a contiguous 32*L*2B run
        dma_engs = [nc.sync, nc.scalar, nc.gpsimd]
        for ri, (k0, nwin, L) in enumerate(runs):
            Er = P * L * nwin
            b0 = base[k0]
            nw4 = 4 * nwin
            cp = cppool.tile([HC + 4, ERUN], b16, tag="cp")
            for k in range(k0, k0 + nwin):
                E = P * L
                bw = base[k]
                off = bw - b0
                xs = xpool.tile([D_IN, Emax], b16, tag="xs")
                dma_engs[k % 3].dma_start(xs[:, :E], xsrcT[:, bw:bw + E])
                eat = epool.tile([ED_DIM + 1, Emax], b16, tag="eat")
                dma_engs[(k + 1) % 3].dma_start(eat[:, :E],
                                                eaTx[:, bw:bw + E])
                ng = math.ceil(E / 512)
                for g in range(ng):
                    c0 = g * 512
                    c1 = min(E, c0 + 512)
                    ps = pspool.tile([HC + 4, 512], f32)
                    nc.tensor.matmul(out=ps[:, :c1 - c0], lhsT=W68_sb[:],
                                     rhs=xs[:, c0:c1], start=True, stop=False)
                    nc.tensor.matmul(out=ps[:, :c1 - c0], lhsT=vTe_sb[:],
                                     rhs=eat[:, c0:c1], start=False, stop=True)
                    nc.scalar.activation(cp[:, off + c0:off + c1],
                                         ps[:, :c1 - c0],
                                         mybir.ActivationFunctionType.Copy)
                nc.gpsimd.dma_start(araw_s[:, bw:bw + E],
                                    cp[HC:HC + 4, off:off + E])

            emc = 32 * L   # em free columns; partitions (h, ws)
            nws = 4 * nwin
            aem = empool.tile([P, 32 * Lmax], b16, tag="aem")
            for h in range(H_HEADS):
                dma_engs[h % 3].dma_start(
                    aem[32 * h:32 * h + nws, :emc]
                    .rearrange("w e -> w e"),
                    araw_s[h, b0:b0 + Er].rearrange("(w e) -> w e", e=emc))
            a2 = empool.tile([P, 32 * Lmax], f32, tag="a2")
            nc.vector.tensor_tensor(
                out=a2[:, :emc].rearrange("p (q l) -> p q l", l=L),
                in0=aem[:, :emc].rearrange("p (q l) -> p q l", l=L),
                in1=adst_sb[:, 32 * ri:32 * ri + 32].unsqueeze(2)
                .broadcast_to([P, 32, L]),
                op=mybir.AluOpType.add)
            # lrelu(x) = max(0.2*x, x)
            nc.vector.scalar_tensor_tensor(
                out=a2[:, :emc], in0=a2[:, :emc], scalar=NEG_SLOPE,
                in1=a2[:, :emc], op0=mybir.AluOpType.mult,
                op1=mybir.AluOpType.max)
            wem = empool.tile([P, 32 * Lmax], b16, tag="wem")
            nc.scalar.activation(wem[:, :emc], a2[:, :emc],
                                 mybir.ActivationFunctionType.Exp)
            nc.vector.tensor_reduce(
                out=den_all[:, 32 * ri:32 * ri + 32],
                in_=wem[:, :emc].rearrange("p (q l) -> p q l", l=L),
                axis=mybir.AxisListType.X, op=mybir.AluOpType.add)
            for h in range(H_HEADS):
                dma_engs[(h + 1) % 3].dma_start(
                    w_s[h, b0:b0 + Er].rearrange("(w e) -> w e", e=emc),
                    wem[32 * h:32 * h + nws, :emc])
            wfm = wfpool.tile([4, ERUN], b16, tag="wfm")
            dma_engs[(ri + 2) % 3].dma_start(wfm[:, :Er], w_s[:, b0:b0 + Er])
            wb = wbpool.tile([HC, ERUN], b16, tag="wb")
            wbv = wb[:].rearrange("(h c) e -> h c e", c=C_OUT)
            for ci in range(C_OUT):
                dma_engs[ci % 3].dma_start(wbv[:, ci, :Er], wfm[:, :Er])
            for k in range(k0, k0 + nwin):
                off = base[k] - b0
                for s in range(4):
                    slr = slice(off + 32 * L * s, off + 32 * L * (s + 1))
                    M = mpool.tile([HC, 32 * Lmax], b16, tag="M")
                    nc.vector.tensor_tensor(out=M[:, :32 * L],
                                            in0=cp[:HC, slr], in1=wb[:, slr],
                                            op=mybir.AluOpType.mult)
                    nc.vector.tensor_reduce(
                        out=num_all[:, P * k + 32 * s:P * k + 32 * (s + 1)],
                        in_=M[:, :32 * L].rearrange("p (q l) -> p q l", l=L),
                        axis=mybir.AxisListType.X, op=mybir.AluOpType.add)

        # ---- close: out = (num + es*xh_self) / (den + es) ----
        nc.vector.tensor_tensor(out=den_all[:], in0=den_all[:], in1=esq_sb[:],
                                op=mybir.AluOpType.add)
        nc.vector.reciprocal(den_all[:], den_all[:])
        for ri, (k0, nwin, L) in enumerate(runs):
            for h in range(H_HEADS):
                dma_engs[(h + ri) % 3].dma_start(
                    r_s[h, P * k0:P * (k0 + nwin)]
                    .rearrange("(w q) -> w q", q=32),
                    den_all[32 * h:32 * h + 4 * nwin,
                            32 * ri:32 * ri + 32])
        rb = cpool.tile([HC, NSLOT], f32)
        rbv = rb[:].rearrange("(h c) e -> h c e", c=C_OUT)
        for ci in range(C_OUT):
            dma_engs[ci % 3].dma_start(rbv[:, ci, :], r_s[:, :])
        nc.vector.tensor_tensor(out=num_all[:], in0=num_all[:], in1=exh_sb[:],
                                op=mybir.AluOpType.add)
        nc.vector.tensor_tensor(out=num_all[:], in0=num_all[:], in1=rb[:],
                                op=mybir.AluOpType.mult)
        nc.gpsimd.dma_start(out[:], num_all[:])

    nc.compile()
    return nc


_NC_CACHE = {}


def _get_nc(Ls):
    if Ls not in _NC_CACHE:
        _NC_CACHE[Ls] = _build_nc(Ls)
    return _NC_CACHE[Ls]


def kernel(**inputs):
    x = np.asarray(inputs["x"], dtype=np.float32)
    ei = np.asarray(inputs["edge_index"])
    ea = np.asarray(inputs["edge_attr"], dtype=np.float32)
    W = np.asarray(inputs["W"], dtype=np.float32)
    W_edge = np.asarray(inputs["W_edge"], dtype=np.float32)
    att_src = np.asarray(inputs["att_src"], dtype=np.float32)
    att_dst = np.asarray(inputs["att_dst"], dtype=np.float32)
    att_edge = np.asarray(inputs["att_edge"], dtype=np.float32)
    bias = np.asarray(inputs["bias"], dtype=np.float32)
    src = ei[0].astype(np.int64)
    dst = ei[1].astype(np.int64)

    Ls, in_maps, meta = _prep(x, src, dst, ea, W, W_edge, att_src, att_dst,
                              att_edge)
    nc = _get_nc(Ls)

    from concourse.bass_utils import run_bass_kernel_spmd
    res = run_bass_kernel_spmd(nc, in_maps, core_ids=list(range(NCORES)),
                               trace=TRACE)
    if TRACE:
        global LAST_RESULT
        LAST_RESULT = res

    n = x.shape[0]
    out = np.empty((n, HC), np.float32)
    nk, ns, ncore = meta["node_k"], meta["node_slot"], meta["node_core"]
    for c in range(NCORES):
        sel = ncore == c
        out[sel] = res.results[c]["out"][:, nk[sel] * P + ns[sel]].T
    return (out + bias[None, :]).astype(np.float32)


# revision 10
# speedup vs baseline: 1.5911x; 1.5141x over previous
"""GAT message-passing kernel for Trainium2, 8 NeuronCores — V3 (gather-free).

Layout strategy (dst-sharded, feature-major, host-pregathered):
 - Nodes sorted by in-degree and packed into 392 windows of 128; window k of
   every core uses one shared edge capacity L_k (max in-degree among the 8
   cores' k-th windows, degree-sorted so padding is tiny), making the SPMD
   program identical across cores.
 - Edge slot (window k, dst slot q, lane l) lives at column
   128*prefix(k) + q*L_k + l.  The host ships, per core, the PRE-GATHERED
   source features x[src_e].T (bf16 [128, S]) and edge features
   [edge_attr | pad_flag].T (bf16 [65, S]) in slot order, so the device
   performs NO per-edge dma_gather (the baseline's 925us gpsimd cost) —
   per-edge projections run as stationary-weight bf16 matmuls at
   1 cycle/column on the PE.
 - Per window: PSUM[68, <=512] accumulates two matmuls:
     rows 0:64  xh_src = W @ x_src            (lhsT = [W.T | u_src.T])
     rows 64:68 a_src + a_edge + pad_flag     (lhsT = vT_ext over [ea | flag])
   a_dst is added in "em" space ([128 = 4h x 32 q, L] tiles reached via a
   DRAM-hop shuffle) where lrelu+exp run full-lane; softmax denominators are
   an X-axis reduce; exp weights return via DRAM to feature-major rows and
   are row-replicated (16 DMAs) for the message multiply; the weighted
   message aggregation per dst node is an X-axis segmented reduce.
 - Per-node self-loop terms (PyG GATConv: self edge_attr = per-dst mean of
   incoming edge_attr) are host-precomputed: es = exp(lrelu(alpha_self)) and
   es*xh fold in at the end.  Output is feature-major [64, 6272] per core;
   the host transposes and scatters back to node order.
"""

import math

import numpy as np
import ml_dtypes

NCORES = 8
P = 128
D_IN = 128
H_HEADS = 4
C_OUT = 16
HC = 64
ED_DIM = 64
NEG_SLOPE = 0.2
PAD_FLAG = -10000.0

TRACE = False
LAST_RESULT = None

bf16 = ml_dtypes.bfloat16


ERUN = 6144
NWIN_RUN_MAX = 8


def _runs_of(Ls):
    runs = []
    k = 0
    NW = len(Ls)
    while k < NW:
        L = Ls[k]
        nwin = 1
        while (k + nwin < NW and Ls[k + nwin] == L and nwin < NWIN_RUN_MAX
               and P * L * (nwin + 1) <= ERUN):
            nwin += 1
        runs.append((k, nwin, L))
        k += nwin
    return runs


def _fold_weights(W, W_edge, att_src, att_dst, att_edge):
    H, C = att_src.shape
    D = W.shape[1]
    ED = W_edge.shape[1]
    u_src = np.einsum("hc,hcd->hd", att_src, W.reshape(H, C, D))
    u_dst = np.einsum("hc,hcd->hd", att_dst, W.reshape(H, C, D))
    v = np.einsum("hc,hcd->hd", att_edge, W_edge.reshape(H, C, ED))
    W68 = np.zeros((D, HC + H), np.float32)
    W68[:, :HC] = W.T
    W68[:, HC:] = u_src.T
    vTe = np.zeros((ED + 1, HC + H), np.float32)
    vTe[:ED, HC:] = v.T
    vTe[ED, HC:] = 1.0  # pad-flag passthrough into every head's alpha
    return W68, vTe, u_src, u_dst, v


def _prep(x, src, dst, ea, W, W_edge, att_src, att_dst, att_edge):
    n = x.shape[0]
    W68, vTe, u_src, u_dst, v = _fold_weights(W, W_edge, att_src, att_dst,
                                              att_edge)
    deg = np.bincount(dst, minlength=n).astype(np.int64)
    order = np.argsort(-deg, kind="stable")
    NW = math.ceil(n / (P * NCORES))          # windows per core (49)
    NWIN = NCORES * NW                        # total windows (392)
    NSLOT = NW * P                            # nodes per core (6272)

    # node -> (sorted-window, slot); sorted-window w -> (core w%8, pos w//8)
    pos_of = np.full(NWIN * P, -1, np.int64)  # padded node list in order
    pos_of[: n] = order
    swin = np.arange(NWIN * P) // P
    # L per sorted window = max degree inside = degree of its first node
    L_raw = np.maximum(deg[order[(np.arange(NWIN) * P).clip(max=n - 1)]], 1)
    # unified per-position L: window position k uses L of sorted window 8k
    Ls = tuple(int(L_raw[NCORES * k]) for k in range(NW))
    S = P * sum(Ls)
    base = np.zeros(NW + 1, np.int64)
    np.cumsum([P * L for L in Ls], out=base[1:])

    node_swin = np.empty(n, np.int64)
    node_slot = np.empty(n, np.int64)
    inv = np.argsort(order, kind="stable")    # node -> rank in sorted order
    node_swin = inv // P
    node_slot = inv % P
    node_core = node_swin % NCORES
    node_k = node_swin // NCORES

    # edge lane index within its dst node
    E = src.shape[0]
    eorder = np.argsort(dst, kind="stable")
    counts = np.bincount(dst, minlength=n)
    offs = np.zeros(n + 1, np.int64)
    np.cumsum(counts, out=offs[1:])
    lane = np.empty(E, np.int64)
    lane[eorder] = np.arange(E) - offs[dst[eorder]]

    ek = node_k[dst]
    ecore = node_core[dst]
    Ls_arr = np.array(Ls, np.int64)
    ecol = base[ek] + node_slot[dst] * Ls_arr[ek] + lane
    assert (lane < Ls_arr[ek]).all()

    xbf = x.astype(bf16)
    xh = (x @ W.T).astype(np.float32)                       # [N, 64]
    a_src_self = x @ u_src.T
    a_dst_self = x @ u_dst.T
    cnt = np.maximum(deg, 1).astype(np.float32)
    loop_attr = np.zeros((n, ED_DIM), np.float32)
    np.add.at(loop_attr, dst, ea)
    loop_attr /= cnt[:, None]
    a_edge_self = loop_attr @ v.T
    al_self = a_src_self + a_dst_self + a_edge_self
    al_self = np.where(al_self > 0, al_self, NEG_SLOPE * al_self)
    es = np.exp(al_self).astype(np.float32)                 # [N, 4]
    a_dst_n = (x @ u_dst.T).astype(np.float32)              # [N, 4]

    in_maps = []
    for c in range(NCORES):
        em = ecore == c
        cols = ecol[em]
        xsrc_rows = np.zeros((S, D_IN), bf16)
        xsrc_rows[cols] = xbf[src[em]]
        eat = np.zeros((ED_DIM + 1, S), np.float32)
        eat[ED_DIM, :] = PAD_FLAG
        eat[:ED_DIM, cols] = ea[em].T
        eat[ED_DIM, cols] = 0.0

        # node tables in this core's (k, q) order
        nsel = node_core == c
        nid = np.where(nsel)[0]
        j = node_k[nid] * P + node_slot[nid]                # col 0..NSLOT-1
        runs = _runs_of(Ls)
        run_of_k = np.empty(NW, np.int64)
        ws0_of_k = np.empty(NW, np.int64)
        for r, (k0, nwin, L) in enumerate(runs):
            run_of_k[k0:k0 + nwin] = r
            ws0_of_k[k0:k0 + nwin] = 4 * (np.arange(nwin))
        NR = len(runs)
        esq = np.ones((P, 32 * NR), np.float32)             # pads: es=1
        adq = np.zeros((P, 32 * NR), bf16)
        exh = np.zeros((HC, NSLOT), bf16)
        q32 = j % 32
        sg = (j // 32) % 4
        k_ = j // P
        r_ = run_of_k[k_]
        ws = ws0_of_k[k_] + sg
        for h in range(H_HEADS):
            esq[h * 32 + ws, 32 * r_ + q32] = es[nid, h]
            adq[h * 32 + ws, 32 * r_ + q32] = a_dst_n[nid, h].astype(bf16)
        exh[:, j] = (es[nid].repeat(C_OUT, axis=1) * xh[nid]).T.astype(bf16)
        in_maps.append(dict(
            xsrcT=np.ascontiguousarray(xsrc_rows.T),
            eaTx=eat.astype(bf16),
            adstqm=adq,
            esqm=esq,
            esxh=exh,
            W68=W68.astype(bf16),
            vTe=vTe.astype(bf16),
        ))
    meta = dict(node_core=node_core, node_k=node_k, node_slot=node_slot)
    return Ls, in_maps, meta


def _build_nc(Ls):
    import concourse.bass as bass
    import concourse.tile as tile
    from concourse import bacc, mybir
    from contextlib import ExitStack

    f32 = mybir.dt.float32
    b16 = mybir.dt.bfloat16
    NW = len(Ls)
    S = P * sum(Ls)
    NSLOT = NW * P
    Lmax = Ls[0]
    Emax = P * Lmax
    base = [0]
    for L in Ls:
        base.append(base[-1] + P * L)

    nc = bacc.Bacc("TRN2", target_bir_lowering=False, debug=False,
                   num_devices=NCORES)
    xsrcT = nc.dram_tensor("xsrcT", [D_IN, S], b16, kind="ExternalInput").ap()
    eaTx = nc.dram_tensor("eaTx", [ED_DIM + 1, S], b16,
                          kind="ExternalInput").ap()
    runs = _runs_of(Ls)
    NR = len(runs)
    adstqm = nc.dram_tensor("adstqm", [P, 32 * NR], b16,
                            kind="ExternalInput").ap()
    esqm = nc.dram_tensor("esqm", [P, 32 * NR], f32,
                          kind="ExternalInput").ap()
    esxh = nc.dram_tensor("esxh", [HC, NSLOT], b16, kind="ExternalInput").ap()
    W68 = nc.dram_tensor("W68", [D_IN, HC + 4], b16, kind="ExternalInput").ap()
    vTe = nc.dram_tensor("vTe", [ED_DIM + 1, HC + 4], b16,
                         kind="ExternalInput").ap()
    out = nc.dram_tensor("out", [HC, NSLOT], f32, kind="ExternalOutput").ap()
    araw_s = nc.dram_tensor("araw_s", [4, S], b16).ap()
    w_s = nc.dram_tensor("w_s", [4, S], b16).ap()
    r_s = nc.dram_tensor("r_s", [4, NSLOT], f32).ap()

    with tile.TileContext(nc) as tc, ExitStack() as ctx:
        cpool = ctx.enter_context(tc.tile_pool(name="const", bufs=1))
        xpool = ctx.enter_context(tc.tile_pool(name="xin", bufs=2))
        epool = ctx.enter_context(tc.tile_pool(name="eain", bufs=2))
        cppool = ctx.enter_context(tc.tile_pool(name="cp", bufs=2))
        wbpool = ctx.enter_context(tc.tile_pool(name="wb", bufs=2))
        wfpool = ctx.enter_context(tc.tile_pool(name="wf", bufs=2))
        empool = ctx.enter_context(tc.tile_pool(name="em", bufs=4))
        mpool = ctx.enter_context(tc.tile_pool(name="m", bufs=2))
        pspool = ctx.enter_context(tc.tile_pool(name="ps", bufs=8,
                                                space="PSUM"))

        W68_sb = cpool.tile([D_IN, HC + 4], b16)
        nc.sync.dma_start(W68_sb[:], W68[:])
        vTe_sb = cpool.tile([ED_DIM + 1, HC + 4], b16)
        nc.sync.dma_start(vTe_sb[:], vTe[:])
        adst_sb = cpool.tile([P, 32 * NR], b16)
        nc.sync.dma_start(adst_sb[:], adstqm[:])
        esq_sb = cpool.tile([P, 32 * NR], f32)
        nc.gpsimd.dma_start(esq_sb[:], esqm[:])
        exh_sb = cpool.tile([HC, NSLOT], b16)
        nc.scalar.dma_start(exh_sb[:], esxh[:])
        den_all = cpool.tile([P, 32 * NR], f32)
        num_all = cpool.tile([HC, NSLOT], f32)

        # equal-L runs of consecutive windows (shared with _prep): em space
        # puts (head, window-subgroup) on partitions and (q, l) on the free
        # axis so every shuffle descriptor moves a contiguous 32*L*2B run
        dma_engs = [nc.sync, nc.scalar, nc.gpsimd]
        for ri, (k0, nwin, L) in enumerate(runs):
            Er = P * L * nwin
            b0 = base[k0]
            nw4 = 4 * nwin
            cp = cppool.tile([HC + 4, ERUN], b16, tag="cp")
            for k in range(k0, k0 + nwin):
                E = P * L
                bw = base[k]
                off = bw - b0
                xs = xpool.tile([D_IN, Emax], b16, tag="xs")
                dma_engs[k % 3].dma_start(xs[:, :E], xsrcT[:, bw:bw + E])
                eat = epool.tile([ED_DIM + 1, Emax], b16, tag="eat")
                dma_engs[(k + 1) % 3].dma_start(eat[:, :E],
                                                eaTx[:, bw:bw + E])
                ng = math.ceil(E / 512)
                for g in range(ng):
                    c0 = g * 512
                    c1 = min(E, c0 + 512)
                    ps = pspool.tile([HC + 4, 512], f32)
                    nc.tensor.matmul(out=ps[:, :c1 - c0], lhsT=W68_sb[:],
                                     rhs=xs[:, c0:c1], start=True, stop=False)
                    nc.tensor.matmul(out=ps[:, :c1 - c0], lhsT=vTe_sb[:],
                                     rhs=eat[:, c0:c1], start=False, stop=True)
                    nc.scalar.activation(cp[:, off + c0:off + c1],
                                         ps[:, :c1 - c0],
                                         mybir.ActivationFunctionType.Copy)
                nc.gpsimd.dma_start(araw_s[:, bw:bw + E],
                                    cp[HC:HC + 4, off:off + E])

            emc = 32 * L   # em free columns; partitions (h, ws)
            nws = 4 * nwin
            aem = empool.tile([P, 32 * Lmax], b16, tag="aem")
            for h in range(H_HEADS):
                dma_engs[h % 3].dma_start(
                    aem[32 * h:32 * h + nws, :emc]
                    .rearrange("w e -> w e"),
                    araw_s[h, b0:b0 + Er].rearrange("(w e) -> w e", e=emc))
            a2 = empool.tile([P, 32 * Lmax], f32, tag="a2")
            nc.vector.tensor_tensor(
                out=a2[:, :emc].rearrange("p (q l) -> p q l", l=L),
                in0=aem[:, :emc].rearrange("p (q l) -> p q l", l=L),
                in1=adst_sb[:, 32 * ri:32 * ri + 32].unsqueeze(2)
                .broadcast_to([P, 32, L]),
                op=mybir.AluOpType.add)
            # lrelu(x) = max(0.2*x, x)
            nc.vector.scalar_tensor_tensor(
                out=a2[:, :emc], in0=a2[:, :emc], scalar=NEG_SLOPE,
                in1=a2[:, :emc], op0=mybir.AluOpType.mult,
                op1=mybir.AluOpType.max)
            wem = empool.tile([P, 32 * Lmax], b16, tag="wem")
            nc.scalar.activation(wem[:, :emc], a2[:, :emc],
                                 mybir.ActivationFunctionType.Exp)
            nc.vector.tensor_reduce(
                out=den_all[:, 32 * ri:32 * ri + 32],
                in_=wem[:, :emc].rearrange("p (q l) -> p q l", l=L),
                axis=mybir.AxisListType.X, op=mybir.AluOpType.add)
            for h in range(H_HEADS):
                dma_engs[(h + 1) % 3].dma_start(
                    w_s[h, b0:b0 + Er].rearrange("(w e) -> w e", e=emc),
                    wem[32 * h:32 * h + nws, :emc])
            wb = wbpool.tile([HC, ERUN], b16, tag="wb")
            wbv = wb[:].rearrange("(h c) e -> h c e", c=C_OUT)
            for ci in range(C_OUT):
                dma_engs[ci % 3].dma_start(wbv[:, ci, :Er],
                                           w_s[:, b0:b0 + Er])
            for k in range(k0, k0 + nwin):
                off = base[k] - b0
                for s in range(4):
                    slr = slice(off + 32 * L * s, off + 32 * L * (s + 1))
                    M = mpool.tile([HC, 32 * Lmax], b16, tag="M")
                    nc.vector.tensor_tensor(out=M[:, :32 * L],
                                            in0=cp[:HC, slr], in1=wb[:, slr],
                                            op=mybir.AluOpType.mult)
                    nc.vector.tensor_reduce(
                        out=num_all[:, P * k + 32 * s:P * k + 32 * (s + 1)],
                        in_=M[:, :32 * L].rearrange("p (q l) -> p q l", l=L),
                        axis=mybir.AxisListType.X, op=mybir.AluOpType.add)

        # ---- close: out = (num + es*xh_self) / (den + es) ----
        nc.vector.tensor_tensor(out=den_all[:], in0=den_all[:], in1=esq_sb[:],
                                op=mybir.AluOpType.add)
        nc.vector.reciprocal(den_all[:], den_all[:])
        for ri, (k0, nwin, L) in enumerate(runs):
            for h in range(H_HEADS):
                dma_engs[(h + ri) % 3].dma_start(
                    r_s[h, P * k0:P * (k0 + nwin)]
                    .rearrange("(w q) -> w q", q=32),
                    den_all[32 * h:32 * h + 4 * nwin,
                            32 * ri:32 * ri + 32])
        rb = cpool.tile([HC, NSLOT], f32)
        rbv = rb[:].rearrange("(h c) e -> h c e", c=C_OUT)
        for ci in range(C_OUT):
            dma_engs[ci % 3].dma_start(rbv[:, ci, :], r_s[:, :])
        nc.vector.tensor_tensor(out=num_all[:], in0=num_all[:], in1=exh_sb[:],
                                op=mybir.AluOpType.add)
        nc.vector.tensor_tensor(out=num_all[:], in0=num_all[:], in1=rb[:],
                                op=mybir.AluOpType.mult)
        nc.gpsimd.dma_start(out[:], num_all[:])

    nc.compile()
    return nc


_NC_CACHE = {}


def _get_nc(Ls):
    if Ls not in _NC_CACHE:
        _NC_CACHE[Ls] = _build_nc(Ls)
    return _NC_CACHE[Ls]


def kernel(**inputs):
    x = np.asarray(inputs["x"], dtype=np.float32)
    ei = np.asarray(inputs["edge_index"])
    ea = np.asarray(inputs["edge_attr"], dtype=np.float32)
    W = np.asarray(inputs["W"], dtype=np.float32)
    W_edge = np.asarray(inputs["W_edge"], dtype=np.float32)
    att_src = np.asarray(inputs["att_src"], dtype=np.float32)
    att_dst = np.asarray(inputs["att_dst"], dtype=np.float32)
    att_edge = np.asarray(inputs["att_edge"], dtype=np.float32)
    bias = np.asarray(inputs["bias"], dtype=np.float32)
    src = ei[0].astype(np.int64)
    dst = ei[1].astype(np.int64)

    Ls, in_maps, meta = _prep(x, src, dst, ea, W, W_edge, att_src, att_dst,
                              att_edge)
    nc = _get_nc(Ls)

    from concourse.bass_utils import run_bass_kernel_spmd
    res = run_bass_kernel_spmd(nc, in_maps, core_ids=list(range(NCORES)),
                               trace=TRACE)
    if TRACE:
        global LAST_RESULT
        LAST_RESULT = res

    n = x.shape[0]
    out = np.empty((n, HC), np.float32)
    nk, ns, ncore = meta["node_k"], meta["node_slot"], meta["node_core"]
    for c in range(NCORES):
        sel = ncore == c
        out[sel] = res.results[c]["out"][:, nk[sel] * P + ns[sel]].T
    return (out + bias[None, :]).astype(np.float32)
